# revision 23
# baseline (speedup 1.0000x reference)
"""AttnBlock (GroupNorm + 4-head self-attention + out-proj) on 8 trn2 cores.

Sharding: core = (batch b in 0..1) x (query-quarter qc in 0..3).
Each core computes the full pipeline for its batch and its 1024-query slice:
  - GroupNorm folded into the QKV weights (per-channel scale/bias).
  - kT/qT stacked as [(head, d)=128, S] so K=32 attention matmuls row-tile
    across the 4 heads (4 concurrent 32-row PE groups).
  - AV uses col-tiling (M=33: 32 v-dims + a ones column that yields the
    softmax denominator for free) with head pairs at PSUM partitions 0/64.
  - Softmax normalization, v-bias and out-proj bias are folded into the
    final projection epilogue.
Output slices are disjoint, so the host just concatenates 8 results.
"""

import numpy as np
from contextlib import ExitStack

import concourse.bass as bass
import concourse.mybir as mybir
import concourse.tile as tile
from concourse import bacc
from concourse.bass_utils import run_bass_kernel_spmd

F32 = mybir.dt.float32
AF = mybir.ActivationFunctionType
ALU = mybir.AluOpType
AX = mybir.AxisListType

HEADS, DH = 4, 32
C = 128           # channels == HEADS*DH
S = 4096          # spatial f*h*w
IC = 1024         # queries per core
NJT = S // 128    # 32 key tiles
SCALE = DH ** -0.5
EPS = 1e-5
NG = 32           # groupnorm groups
GSIZE = (C // NG) * S  # elements per group


def _build():
    nc = bacc.Bacc("TRN2", target_bir_lowering=False)
    d_xb = nc.declare_dram_parameter("xb", [C, S], F32, isOutput=False)
    d_xq = nc.declare_dram_parameter("xq", [C, IC], F32, isOutput=False)
    d_wqkv = nc.declare_dram_parameter("wqkvT", [C, 3 * C], F32, isOutput=False)
    d_wout = nc.declare_dram_parameter("woutT", [C, C], F32, isOutput=False)
    d_gam = nc.declare_dram_parameter("gam", [C, 1], F32, isOutput=False)
    d_bet = nc.declare_dram_parameter("bet", [C, 1], F32, isOutput=False)
    d_bout = nc.declare_dram_parameter("bout", [C, 1], F32, isOutput=False)
    d_gmap = nc.declare_dram_parameter("gmap", [C, NG], F32, isOutput=False)
    d_gmapT = nc.declare_dram_parameter("gmapT", [NG, C], F32, isOutput=False)
    d_bmap = nc.declare_dram_parameter("bmap", [C, C], F32, isOutput=False)
    d_y = nc.declare_dram_parameter("y", [C, IC], F32, isOutput=True)

    with tile.TileContext(nc) as tc, ExitStack() as ctx:
        nv, ns, nt = nc.vector, nc.scalar, nc.tensor
        P = ctx.enter_context(tc.tile_pool(name="persist", bufs=1))
        EP = ctx.enter_context(tc.tile_pool(name="epool", bufs=4))

        # ---------------- loads ----------------
        # every tensor a matmul reads is bounced once through DVE so PE
        # instructions only ever wait on one proc (LDW codegen allows a
        # single sync-wait); DMAs are chunked so the copies pipeline.
        xb_raw = P.tile([C, S], F32, tag="xb_raw")
        xb = P.tile([C, S], F32, tag="xb")
        for chk in range(8):
            sl = slice(chk * 512, (chk + 1) * 512)
            nc.sync.dma_start(xb_raw[:, sl], d_xb[:, sl])
            nv.tensor_copy(xb[:, sl], xb_raw[:, sl])
        xq_raw = P.tile([C, IC], F32, tag="xq_raw")
        xq = P.tile([C, IC], F32, tag="xq")
        for chk in range(2):
            sl = slice(chk * 512, (chk + 1) * 512)
            nc.sync.dma_start(xq_raw[:, sl], d_xq[:, sl])
            nv.tensor_copy(xq[:, sl], xq_raw[:, sl])
        # weights used as matmul lhsT bounce through a DVE copy so consuming
        # matmuls only wait on DVE (+PE); 3 sync-waits overflow LDW codegen
        wq_raw = P.tile([C, 3 * C], F32, tag="wq_raw")
        nc.sync.dma_start(wq_raw[:], d_wqkv[:])
        wq = P.tile([C, 3 * C], F32, tag="wq")
        nv.tensor_copy(wq[:], wq_raw[:])
        wo_raw = P.tile([C, C], F32, tag="wo_raw")
        nc.sync.dma_start(wo_raw[:], d_wout[:])
        wo = P.tile([C, C], F32, tag="wo")
        nv.tensor_copy(wo[:], wo_raw[:])
        gbb_raw = P.tile([C, 3], F32, tag="gbb_raw")
        nc.sync.dma_start(gbb_raw[:, 0:1], d_gam[:])
        nc.sync.dma_start(gbb_raw[:, 1:2], d_bet[:])
        nc.sync.dma_start(gbb_raw[:, 2:3], d_bout[:])
        gbb = P.tile([C, 3], F32, tag="gbb")
        nv.tensor_copy(gbb[:, 0:1], gbb_raw[:, 0:1])
        nv.tensor_copy(gbb[:, 1:2], gbb_raw[:, 1:2])
        nv.tensor_copy(gbb[:, 2:3], gbb_raw[:, 2:3])
        gam, bet, bout = gbb[:, 0:1], gbb[:, 1:2], gbb[:, 2:3]
        gmap_raw = P.tile([C, NG], F32, tag="gmap_raw")
        nc.sync.dma_start(gmap_raw[:], d_gmap[:])
        gmap = P.tile([C, NG], F32, tag="gmap")
        nv.tensor_copy(gmap[:], gmap_raw[:])
        gmapT_raw = P.tile([NG, C], F32, tag="gmapT_raw")
        nc.sync.dma_start(gmapT_raw[:], d_gmapT[:])
        gmapT = P.tile([NG, C], F32, tag="gmapT")
        nv.tensor_copy(gmapT[:], gmapT_raw[:])
        bmap_raw = P.tile([C, C], F32, tag="bmap_raw")
        nc.sync.dma_start(bmap_raw[:], d_bmap[:])
        bmap = P.tile([C, C], F32, tag="bmap")
        nv.tensor_copy(bmap[:], bmap_raw[:])

        # persistent products
        kT = P.tile([C, S], F32, tag="kT")        # [(h,d), j]
        qT = P.tile([C, IC], F32, tag="qT")       # [(h,d), i]
        vaug = P.tile([C, NJT * HEADS * (DH + 1)], F32, tag="vaug")
        vaug3 = vaug[:].rearrange("p (a b) -> p a b", b=DH + 1)  # a = jt*4+h
        wqs = P.tile([C, 3 * C], F32, tag="wqs")
        stat = P.tile([C, 4], F32, tag="stat")
        gstat = P.tile([NG, 12], F32, tag="gstat")
        qb = P.tile([C, 1], F32, tag="qb")
        kb = P.tile([C, 1], F32, tag="kb")
        vb = P.tile([C, 1], F32, tag="vb")
        ybias = P.tile([C, 1], F32, tag="ybias")
        osc = P.tile([C, IC], F32, tag="osc")
        ysb = P.tile([C, IC], F32, tag="ysb")
        d4 = P.tile([C, IC], F32, tag="d4")
        r4 = P.tile([C, IC], F32, tag="r4")

        # ---------------- prologue: GN stats + weight folding + qkv ----
        with tc.tile_pool(name="pps", bufs=2, space="PSUM") as PPS, \
             tc.tile_pool(name="wsc", bufs=1) as WSC:
            # per-channel sum and sum of squares
            nv.tensor_reduce(stat[:, 0:1], xb[:], AX.X, ALU.add)
            xsq = WSC.tile([C, S], F32, tag="xsq")
            nv.tensor_mul(xsq[:], xb[:], xb[:])
            nv.tensor_reduce(stat[:, 1:2], xsq[:], AX.X, ALU.add)
            # group-combine via matmul with group indicator matrix
            gs_p = PPS.tile([NG, 2], F32, tag="tiny")
            nt.matmul(gs_p[:], gmap[:], stat[:, 0:2], start=True, stop=True)
            m_g = gstat[:, 0:1]
            ex2 = gstat[:, 1:2]
            nv.tensor_scalar_mul(m_g, gs_p[:, 0:1], 1.0 / GSIZE)
            nv.tensor_scalar_mul(ex2, gs_p[:, 1:2], 1.0 / GSIZE)
            msq = gstat[:, 2:3]
            nv.tensor_mul(msq, m_g, m_g)
            vare = gstat[:, 3:4]
            nv.tensor_sub(vare, ex2, msq)
            nv.tensor_scalar_add(vare, vare, EPS)   # var + eps
            sq = gstat[:, 4:5]
            ns.activation(sq, vare, AF.Sqrt)
            r = gstat[:, 5:6]
            nv.reciprocal(r, sq)
            # two Newton steps: r <- r * (1.5 - 0.5 * vare * r^2)
            for it in range(2):
                t1 = gstat[:, 6:7]
                nv.tensor_mul(t1, r, r)
                t2 = gstat[:, 7:8]
                nv.tensor_mul(t2, t1, vare)
                t3 = gstat[:, 8:9]
                nv.tensor_scalar(t3, t2, -0.5, 1.5, ALU.mult, ALU.add)
                rn = gstat[:, 9:10] if it == 0 else gstat[:, 10:11]
                nv.tensor_mul(rn, r, t3)
                r = rn
            # broadcast group mean/rstd back to channels
            st2 = gstat[:, 6:8]
            nv.tensor_copy(st2[:, 0:1], m_g)
            nv.tensor_copy(st2[:, 1:2], r)
            ch_p = PPS.tile([C, 2], F32, tag="tiny")
            nt.matmul(ch_p[:], gmapT[:], st2, start=True, stop=True)
            scale_c = stat[:, 2:3]
            nv.tensor_mul(scale_c, ch_p[:, 1:2], gam)
            tb = stat[:, 3:4]
            nv.tensor_mul(tb, ch_p[:, 0:1], scale_c)
            nv.tensor_sub(tb, bet, tb)

            # fold GN scale into qkv weights; compute qkv biases from GN shift
            nv.tensor_scalar_mul(wqs[:], wq[:], scale_c)
            for bi, btile in enumerate((qb, kb, vb)):
                bp = PPS.tile([C, 1], F32, tag="tiny")
                nt.matmul(bp[:], wq[:, bi * C:(bi + 1) * C], tb, start=True, stop=True)
                nv.tensor_copy(btile[:], bp[:])
            ybp = PPS.tile([C, 1], F32, tag="tiny")
            nt.matmul(ybp[:], wo[:], vb[:], start=True, stop=True)
            nv.tensor_add(ybias[:], ybp[:], bout)

            # v first (so the later kT/qT DVE ticks cover vaug for the
            # attention loop's AV matmuls), in [j, (h,d)] layout with ones
            # columns interleaved (softmax denominator)
            nv.memset(vaug3[:, :, DH:DH + 1], 1.0)
            for jt in range(NJT):
                pv = PPS.tile([C, C], F32, tag="pv")
                nt.matmul(pv[:], xb[:, jt * 128:(jt + 1) * 128], wqs[:, 2 * C:3 * C],
                          start=True, stop=True)
                nv.tensor_copy(vaug3[:, jt * HEADS:(jt + 1) * HEADS, 0:DH],
                               pv[:].rearrange("p (h d) -> p h d", d=DH))
            # qT / kT (with folded bias), stacked [(h,d), *]
            for chk in range(IC // 512):
                pq = PPS.tile([C, 512], F32, tag="pq")
                nt.matmul(pq[:], wqs[:, 0:C], xq[:, chk * 512:(chk + 1) * 512],
                          start=True, stop=True)
                nv.tensor_scalar_add(qT[:, chk * 512:(chk + 1) * 512], pq[:], qb[:])
            for chk in range(S // 512):
                pk = PPS.tile([C, 512], F32, tag="pq")
                nt.matmul(pk[:], wqs[:, C:2 * C], xb[:, chk * 512:(chk + 1) * 512],
                          start=True, stop=True)
                nv.tensor_scalar_add(kT[:, chk * 512:(chk + 1) * 512], pk[:], kb[:])

        # ---------------- attention ----------------
        with tc.tile_pool(name="psc", bufs=2, space="PSUM") as PSC, \
             tc.tile_pool(name="po", bufs=2, space="PSUM") as PO:
            o01 = PO.tile([C, IC], F32, tag="o")
            o23 = PO.tile([C, IC], F32, tag="o")
            for jt in range(NJT):
                for h in range(HEADS):
                    sc = PSC.tile([C, IC], F32, tag="sc")
                    for hf in range(2):
                        nt.matmul(sc[:, hf * 512:(hf + 1) * 512],
                                  kT[32 * h:32 * (h + 1), jt * 128:(jt + 1) * 128],
                                  qT[32 * h:32 * (h + 1), hf * 512:(hf + 1) * 512],
                                  start=True, stop=True,
                                  tile_position=(32 * h, 0))
                    e = EP.tile([C, IC], F32, tag="E")
                    ns.activation(e[:], sc[:], AF.Exp, scale=SCALE)
                    o = o01 if h < 2 else o23
                    base = 0 if h % 2 == 0 else 64
                    for hf in range(2):
                        nt.matmul(o[base:base + DH + 1, hf * 512:(hf + 1) * 512],
                                  vaug3[:, jt * HEADS + h, :],
                                  e[:, hf * 512:(hf + 1) * 512],
                                  start=(jt == 0), stop=(jt == NJT - 1),
                                  skip_group_check=True,
                                  tile_position=(0, base))

            # ---------------- epilogue ----------------
            # denominators land at partitions 0/32/64/96 (32-aligned);
            # remaining rows stay 1.0 so reciprocal/matmul see finite values
            nv.memset(d4[:], 1.0)
            nv.tensor_copy(d4[0:1, :], o01[DH:DH + 1, :])
            nv.tensor_copy(d4[32:33, :], o01[64 + DH:64 + DH + 1, :])
            nv.tensor_copy(d4[64:65, :], o23[DH:DH + 1, :])
            nv.tensor_copy(d4[96:97, :], o23[64 + DH:64 + DH + 1, :])
            nv.reciprocal(r4[:], d4[:])
            rexp = PSC.tile([C, IC], F32, tag="sc")
            for hf in range(2):
                nt.matmul(rexp[:, hf * 512:(hf + 1) * 512], bmap[:],
                          r4[:, hf * 512:(hf + 1) * 512], start=True, stop=True)
            rexp_sb = P.tile([C, IC], F32, tag="rexps")
            nv.tensor_copy(rexp_sb[:], rexp[:])
            nv.tensor_mul(osc[0:32, :], o01[0:32, :], rexp_sb[0:32, :])
            nv.tensor_mul(osc[32:64, :], o01[64:96, :], rexp_sb[32:64, :])
            nv.tensor_mul(osc[64:96, :], o23[0:32, :], rexp_sb[64:96, :])
            nv.tensor_mul(osc[96:128, :], o23[64:96, :], rexp_sb[96:128, :])
            ypsum = PO.tile([C, IC], F32, tag="o")
            for hf in range(2):
                nt.matmul(ypsum[:, hf * 512:(hf + 1) * 512], wo[:],
                          osc[:, hf * 512:(hf + 1) * 512], start=True, stop=True)
            nv.tensor_scalar_add(ysb[:], ypsum[:], ybias[:])
            nc.sync.dma_start(d_y[:], ysb[:])

    nc.compile()   # bacc passes: split sync waits (HW: 1 wait/inst), DCE, regalloc
    return nc


_PROG = None


def _get_prog():
    global _PROG
    if _PROG is None:
        _PROG = _build()
    return _PROG


def _in_maps(x, gn_gamma, gn_beta, w_qkv, w_out, b_out):
    x = np.asarray(x, dtype=np.float32)
    gmap = np.zeros((C, NG), dtype=np.float32)
    gmap[np.arange(C), np.arange(C) // (C // NG)] = 1.0
    bmap = np.zeros((C, C), dtype=np.float32)
    for h in range(HEADS):
        bmap[32 * h, 32 * h:32 * (h + 1)] = 1.0
    base = dict(
        wqkvT=np.ascontiguousarray(np.asarray(w_qkv, np.float32).T),
        woutT=np.ascontiguousarray(np.asarray(w_out, np.float32).T),
        gam=np.asarray(gn_gamma, np.float32).reshape(C, 1),
        bet=np.asarray(gn_beta, np.float32).reshape(C, 1),
        bout=np.asarray(b_out, np.float32).reshape(C, 1),
        gmap=gmap,
        gmapT=np.ascontiguousarray(gmap.T),
        bmap=bmap,
    )
    maps = []
    for core in range(8):
        b, qc = core // 4, core % 4
        xb = np.ascontiguousarray(x[b].reshape(C, S))
        m = dict(base)
        m["xb"] = xb
        m["xq"] = np.ascontiguousarray(xb[:, qc * IC:(qc + 1) * IC])
        maps.append(m)
    return maps


def kernel(x, gn_gamma, gn_beta, w_qkv, w_out, b_out):
    nc = _get_prog()
    maps = _in_maps(x, gn_gamma, gn_beta, w_qkv, w_out, b_out)
    res = run_bass_kernel_spmd(nc, maps, list(range(8))).results
    y = np.empty((2, C, S), dtype=np.float32)
    for core in range(8):
        b, qc = core // 4, core % 4
        y[b, :, qc * IC:(qc + 1) * IC] = res[core]["y"]
    return y.reshape(2, C, 16, 16, 16)


# revision 27
# speedup vs baseline: 1.0526x; 1.0526x over previous
"""AttnBlock (GroupNorm + 4-head self-attention + out-proj) on 8 trn2 cores.

Sharding: core = (batch b in 0..1) x (query-quarter qc in 0..3).
Each core computes the full pipeline for its batch and its 1024-query slice:
  - GroupNorm folded into the QKV weights (per-channel scale/bias).
  - kT/qT stacked as [(head, d)=128, S] so K=32 attention matmuls row-tile
    across the 4 heads (4 concurrent 32-row PE groups).
  - AV uses col-tiling (M=33: 32 v-dims + a ones column that yields the
    softmax denominator for free) with head pairs at PSUM partitions 0/64.
  - Softmax normalization, v-bias and out-proj bias are folded into the
    final projection epilogue.
Output slices are disjoint, so the host just concatenates 8 results.
"""

import numpy as np
from contextlib import ExitStack

import concourse.bass as bass
import concourse.mybir as mybir
import concourse.tile as tile
from concourse import bacc
from concourse.bass_utils import run_bass_kernel_spmd

F32 = mybir.dt.float32
AF = mybir.ActivationFunctionType
ALU = mybir.AluOpType
AX = mybir.AxisListType

HEADS, DH = 4, 32
C = 128           # channels == HEADS*DH
S = 4096          # spatial f*h*w
IC = 1024         # queries per core
NJT = S // 128    # 32 key tiles
SCALE = DH ** -0.5
EPS = 1e-5
NG = 32           # groupnorm groups
GSIZE = (C // NG) * S  # elements per group


def _build():
    nc = bacc.Bacc("TRN2", target_bir_lowering=False)
    d_xb = nc.declare_dram_parameter("xb", [C, S], F32, isOutput=False)
    d_xq = nc.declare_dram_parameter("xq", [C, IC], F32, isOutput=False)
    d_wqkv = nc.declare_dram_parameter("wqkvT", [C, 3 * C], F32, isOutput=False)
    d_wout = nc.declare_dram_parameter("woutT", [C, C], F32, isOutput=False)
    d_gam = nc.declare_dram_parameter("gam", [C, 1], F32, isOutput=False)
    d_bet = nc.declare_dram_parameter("bet", [C, 1], F32, isOutput=False)
    d_bout = nc.declare_dram_parameter("bout", [C, 1], F32, isOutput=False)
    d_gmap = nc.declare_dram_parameter("gmap", [C, NG], F32, isOutput=False)
    d_gmapT = nc.declare_dram_parameter("gmapT", [NG, C], F32, isOutput=False)
    d_bmap = nc.declare_dram_parameter("bmap", [C, C], F32, isOutput=False)
    d_y = nc.declare_dram_parameter("y", [C, IC], F32, isOutput=True)

    with tile.TileContext(nc) as tc, ExitStack() as ctx:
        nv, ns, nt = nc.vector, nc.scalar, nc.tensor
        P = ctx.enter_context(tc.tile_pool(name="persist", bufs=1))
        EP = ctx.enter_context(tc.tile_pool(name="epool", bufs=4))

        # ---------------- loads ----------------
        # every tensor a matmul reads is bounced once through DVE so PE
        # instructions only ever wait on one proc (LDW codegen allows a
        # single sync-wait); DMAs are chunked so the copies pipeline.
        xb_raw = P.tile([C, S], F32, tag="xb_raw")
        xb = P.tile([C, S], F32, tag="xb")
        for chk in range(8):
            sl = slice(chk * 512, (chk + 1) * 512)
            nc.sync.dma_start(xb_raw[:, sl], d_xb[:, sl])
            nv.tensor_copy(xb[:, sl], xb_raw[:, sl])
        xq_raw = P.tile([C, IC], F32, tag="xq_raw")
        xq = P.tile([C, IC], F32, tag="xq")
        for chk in range(2):
            sl = slice(chk * 512, (chk + 1) * 512)
            nc.sync.dma_start(xq_raw[:, sl], d_xq[:, sl])
            nv.tensor_copy(xq[:, sl], xq_raw[:, sl])
        # weights used as matmul lhsT bounce through a DVE copy so consuming
        # matmuls only wait on DVE (+PE); 3 sync-waits overflow LDW codegen
        wq_raw = P.tile([C, 3 * C], F32, tag="wq_raw")
        nc.sync.dma_start(wq_raw[:], d_wqkv[:])
        wq = P.tile([C, 3 * C], F32, tag="wq")
        nv.tensor_copy(wq[:], wq_raw[:])
        wo_raw = P.tile([C, C], F32, tag="wo_raw")
        nc.sync.dma_start(wo_raw[:], d_wout[:])
        wo = P.tile([C, C], F32, tag="wo")
        nv.tensor_copy(wo[:], wo_raw[:])
        gbb_raw = P.tile([C, 3], F32, tag="gbb_raw")
        nc.sync.dma_start(gbb_raw[:, 0:1], d_gam[:])
        nc.sync.dma_start(gbb_raw[:, 1:2], d_bet[:])
        nc.sync.dma_start(gbb_raw[:, 2:3], d_bout[:])
        gbb = P.tile([C, 3], F32, tag="gbb")
        nv.tensor_copy(gbb[:, 0:1], gbb_raw[:, 0:1])
        nv.tensor_copy(gbb[:, 1:2], gbb_raw[:, 1:2])
        nv.tensor_copy(gbb[:, 2:3], gbb_raw[:, 2:3])
        gam, bet, bout = gbb[:, 0:1], gbb[:, 1:2], gbb[:, 2:3]
        gmap_raw = P.tile([C, NG], F32, tag="gmap_raw")
        nc.sync.dma_start(gmap_raw[:], d_gmap[:])
        gmap = P.tile([C, NG], F32, tag="gmap")
        nv.tensor_copy(gmap[:], gmap_raw[:])
        gmapT_raw = P.tile([NG, C], F32, tag="gmapT_raw")
        nc.sync.dma_start(gmapT_raw[:], d_gmapT[:])
        gmapT = P.tile([NG, C], F32, tag="gmapT")
        nv.tensor_copy(gmapT[:], gmapT_raw[:])
        bmap_raw = P.tile([C, C], F32, tag="bmap_raw")
        nc.sync.dma_start(bmap_raw[:], d_bmap[:])
        bmap = P.tile([C, C], F32, tag="bmap")
        nv.tensor_copy(bmap[:], bmap_raw[:])

        # persistent products
        kT = P.tile([C, S], F32, tag="kT")        # [(h,d), j]
        qT = P.tile([C, IC], F32, tag="qT")       # [(h,d), i]
        vaug = P.tile([C, NJT * HEADS * (DH + 1)], F32, tag="vaug")
        vaug3 = vaug[:].rearrange("p (a b) -> p a b", b=DH + 1)  # a = jt*4+h
        wqs = P.tile([C, 3 * C], F32, tag="wqs")
        stat = P.tile([C, 4], F32, tag="stat")
        stat16 = P.tile([C, 16], F32, tag="stat16")
        gstat = P.tile([NG, 12], F32, tag="gstat")
        qb = P.tile([C, 1], F32, tag="qb")
        kb = P.tile([C, 1], F32, tag="kb")
        vb = P.tile([C, 1], F32, tag="vb")
        ybias = P.tile([C, 1], F32, tag="ybias")
        osc = P.tile([C, IC], F32, tag="osc")
        ysb = P.tile([C, IC], F32, tag="ysb")
        d4 = P.tile([C, IC], F32, tag="d4")
        r4 = P.tile([C, IC], F32, tag="r4")

        # ---------------- prologue: GN stats + weight folding + qkv ----
        with tc.tile_pool(name="pps", bufs=2, space="PSUM") as PPS, \
             tc.tile_pool(name="wsc", bufs=1) as WSC:
            # per-channel sum and sum of squares, chunked to overlap the
            # x DMA (each chunk's reduce runs as soon as its copy lands)
            for chk in range(8):
                sl = slice(chk * 512, (chk + 1) * 512)
                nv.tensor_reduce(stat16[:, chk:chk + 1], xb[:, sl], AX.X, ALU.add)
                xsq = WSC.tile([C, 512], F32, tag="xsq", bufs=2)
                nv.tensor_mul(xsq[:], xb[:, sl], xb[:, sl])
                nv.tensor_reduce(stat16[:, 8 + chk:9 + chk], xsq[:], AX.X, ALU.add)
            nv.tensor_reduce(stat[:, 0:1], stat16[:, 0:8], AX.X, ALU.add)
            nv.tensor_reduce(stat[:, 1:2], stat16[:, 8:16], AX.X, ALU.add)
            # group-combine via matmul with group indicator matrix
            gs_p = PPS.tile([NG, 2], F32, tag="tiny")
            nt.matmul(gs_p[:], gmap[:], stat[:, 0:2], start=True, stop=True)
            m_g = gstat[:, 0:1]
            ex2 = gstat[:, 1:2]
            nv.tensor_scalar_mul(m_g, gs_p[:, 0:1], 1.0 / GSIZE)
            nv.tensor_scalar_mul(ex2, gs_p[:, 1:2], 1.0 / GSIZE)
            msq = gstat[:, 2:3]
            nv.tensor_mul(msq, m_g, m_g)
            vare = gstat[:, 3:4]
            nv.tensor_sub(vare, ex2, msq)
            nv.tensor_scalar_add(vare, vare, EPS)   # var + eps
            sq = gstat[:, 4:5]
            ns.activation(sq, vare, AF.Sqrt)
            r = gstat[:, 5:6]
            nv.reciprocal(r, sq)
            # two Newton steps: r <- r * (1.5 - 0.5 * vare * r^2)
            for it in range(2):
                t1 = gstat[:, 6:7]
                nv.tensor_mul(t1, r, r)
                t2 = gstat[:, 7:8]
                nv.tensor_mul(t2, t1, vare)
                t3 = gstat[:, 8:9]
                nv.tensor_scalar(t3, t2, -0.5, 1.5, ALU.mult, ALU.add)
                rn = gstat[:, 9:10] if it == 0 else gstat[:, 10:11]
                nv.tensor_mul(rn, r, t3)
                r = rn
            # broadcast group mean/rstd back to channels
            st2 = gstat[:, 6:8]
            nv.tensor_copy(st2[:, 0:1], m_g)
            nv.tensor_copy(st2[:, 1:2], r)
            ch_p = PPS.tile([C, 2], F32, tag="tiny")
            nt.matmul(ch_p[:], gmapT[:], st2, start=True, stop=True)
            scale_c = stat[:, 2:3]
            nv.tensor_mul(scale_c, ch_p[:, 1:2], gam)
            tb = stat[:, 3:4]
            nv.tensor_mul(tb, ch_p[:, 0:1], scale_c)
            nv.tensor_sub(tb, bet, tb)

            # fold GN scale into qkv weights; compute qkv biases from GN shift
            nv.tensor_scalar_mul(wqs[:], wq[:], scale_c)
            for bi, btile in enumerate((qb, kb, vb)):
                bp = PPS.tile([C, 1], F32, tag="tiny")
                nt.matmul(bp[:], wq[:, bi * C:(bi + 1) * C], tb, start=True, stop=True)
                nv.tensor_copy(btile[:], bp[:])
            ybp = PPS.tile([C, 1], F32, tag="tiny")
            nt.matmul(ybp[:], wo[:], vb[:], start=True, stop=True)
            nv.tensor_add(ybias[:], ybp[:], bout)

            # qT / kT (with folded bias), stacked [(h,d), *]
            for chk in range(IC // 512):
                pq = PPS.tile([C, 512], F32, tag="pq")
                nt.matmul(pq[:], wqs[:, 0:C], xq[:, chk * 512:(chk + 1) * 512],
                          start=True, stop=True)
                nv.tensor_scalar_add(qT[:, chk * 512:(chk + 1) * 512], pq[:], qb[:])
            for chk in range(S // 512):
                pk = PPS.tile([C, 512], F32, tag="pq")
                nt.matmul(pk[:], wqs[:, C:2 * C], xb[:, chk * 512:(chk + 1) * 512],
                          start=True, stop=True)
                nv.tensor_scalar_add(kT[:, chk * 512:(chk + 1) * 512], pk[:], kb[:])
            # v in [j, (h,d)] layout with ones columns interleaved
            # (softmax denominator); 4 j-tiles batched per PSUM tile
            nv.memset(vaug3[:, :, DH:DH + 1], 1.0)
            for g in range(NJT // 4):
                pv = PPS.tile([C, 512], F32, tag="pq")
                for k in range(4):
                    nt.matmul(pv[:, k * 128:(k + 1) * 128],
                              xb[:, (4 * g + k) * 128:(4 * g + k + 1) * 128],
                              wqs[:, 2 * C:3 * C], start=True, stop=True)
                nv.tensor_copy(vaug3[:, g * 16:(g + 1) * 16, 0:DH],
                               pv[:].rearrange("p (a d) -> p a d", d=DH))

        # ---------------- attention ----------------
        with tc.tile_pool(name="psc", bufs=2, space="PSUM") as PSC, \
             tc.tile_pool(name="po", bufs=2, space="PSUM") as PO:
            o01 = PO.tile([C, IC], F32, tag="o")
            o23 = PO.tile([C, IC], F32, tag="o")
            for jt in range(NJT):
                # head pairs; within a pair the two heads' matmuls are
                # emitted back-to-back on disjoint row/col groups so the PE
                # runs them concurrently (row tiling for QK^T, col tiling
                # for AV)
                for pair in range(2):
                    ha, hb = 2 * pair, 2 * pair + 1
                    sa = PSC.tile([C, IC], F32, tag="sc")
                    sb = PSC.tile([C, IC], F32, tag="sc")
                    for hf in range(2):
                        for h, sc in ((ha, sa), (hb, sb)):
                            nt.matmul(sc[:, hf * 512:(hf + 1) * 512],
                                      kT[32 * h:32 * (h + 1),
                                         jt * 128:(jt + 1) * 128],
                                      qT[32 * h:32 * (h + 1),
                                         hf * 512:(hf + 1) * 512],
                                      start=True, stop=True,
                                      tile_position=(32 * h, 0))
                    ea = EP.tile([C, IC], F32, tag="E")
                    ns.activation(ea[:], sa[:], AF.Exp, scale=SCALE)
                    eb = EP.tile([C, IC], F32, tag="E")
                    ns.activation(eb[:], sb[:], AF.Exp, scale=SCALE)
                    o = o01 if pair == 0 else o23
                    for hf in range(2):
                        for base, e, h in ((0, ea, ha), (64, eb, hb)):
                            nt.matmul(o[base:base + DH + 1,
                                        hf * 512:(hf + 1) * 512],
                                      vaug3[:, jt * HEADS + h, :],
                                      e[:, hf * 512:(hf + 1) * 512],
                                      start=(jt == 0), stop=(jt == NJT - 1),
                                      skip_group_check=True,
                                      tile_position=(0, base))

            # ---------------- epilogue ----------------
            # denominators land at partitions 0/32/64/96 (32-aligned);
            # remaining rows stay 1.0 so reciprocal/matmul see finite values
            nv.memset(d4[:], 1.0)
            nv.tensor_copy(d4[0:1, :], o01[DH:DH + 1, :])
            nv.tensor_copy(d4[32:33, :], o01[64 + DH:64 + DH + 1, :])
            nv.tensor_copy(d4[64:65, :], o23[DH:DH + 1, :])
            nv.tensor_copy(d4[96:97, :], o23[64 + DH:64 + DH + 1, :])
            nv.reciprocal(r4[:], d4[:])
            rexp = PSC.tile([C, IC], F32, tag="sc")
            for hf in range(2):
                nt.matmul(rexp[:, hf * 512:(hf + 1) * 512], bmap[:],
                          r4[:, hf * 512:(hf + 1) * 512], start=True, stop=True)
            rexp_sb = P.tile([C, IC], F32, tag="rexps")
            nv.tensor_copy(rexp_sb[:], rexp[:])
            nv.tensor_mul(osc[0:32, :], o01[0:32, :], rexp_sb[0:32, :])
            nv.tensor_mul(osc[32:64, :], o01[64:96, :], rexp_sb[32:64, :])
            nv.tensor_mul(osc[64:96, :], o23[0:32, :], rexp_sb[64:96, :])
            nv.tensor_mul(osc[96:128, :], o23[64:96, :], rexp_sb[96:128, :])
            ypsum = PO.tile([C, IC], F32, tag="o")
            for hf in range(2):
                nt.matmul(ypsum[:, hf * 512:(hf + 1) * 512], wo[:],
                          osc[:, hf * 512:(hf + 1) * 512], start=True, stop=True)
            nv.tensor_scalar_add(ysb[:], ypsum[:], ybias[:])
            nc.sync.dma_start(d_y[:], ysb[:])

    nc.compile()   # bacc passes: split sync waits (HW: 1 wait/inst), DCE, regalloc
    return nc


_PROG = None


def _get_prog():
    global _PROG
    if _PROG is None:
        _PROG = _build()
    return _PROG


def _in_maps(x, gn_gamma, gn_beta, w_qkv, w_out, b_out):
    x = np.asarray(x, dtype=np.float32)
    gmap = np.zeros((C, NG), dtype=np.float32)
    gmap[np.arange(C), np.arange(C) // (C // NG)] = 1.0
    bmap = np.zeros((C, C), dtype=np.float32)
    for h in range(HEADS):
        bmap[32 * h, 32 * h:32 * (h + 1)] = 1.0
    base = dict(
        wqkvT=np.ascontiguousarray(np.asarray(w_qkv, np.float32).T),
        woutT=np.ascontiguousarray(np.asarray(w_out, np.float32).T),
        gam=np.asarray(gn_gamma, np.float32).reshape(C, 1),
        bet=np.asarray(gn_beta, np.float32).reshape(C, 1),
        bout=np.asarray(b_out, np.float32).reshape(C, 1),
        gmap=gmap,
        gmapT=np.ascontiguousarray(gmap.T),
        bmap=bmap,
    )
    maps = []
    for core in range(8):
        b, qc = core // 4, core % 4
        xb = np.ascontiguousarray(x[b].reshape(C, S))
        m = dict(base)
        m["xb"] = xb
        m["xq"] = np.ascontiguousarray(xb[:, qc * IC:(qc + 1) * IC])
        maps.append(m)
    return maps


def kernel(x, gn_gamma, gn_beta, w_qkv, w_out, b_out):
    nc = _get_prog()
    maps = _in_maps(x, gn_gamma, gn_beta, w_qkv, w_out, b_out)
    res = run_bass_kernel_spmd(nc, maps, list(range(8))).results
    y = np.empty((2, C, S), dtype=np.float32)
    for core in range(8):
        b, qc = core // 4, core % 4
        y[b, :, qc * IC:(qc + 1) * IC] = res[core]["y"]
    return y.reshape(2, C, 16, 16, 16)
